# revision 1
# baseline (speedup 1.0000x reference)
"""Multi-head causal self-attention (B=1, S=4096, D=1024, H=16) on 8 TRN2
NeuronCores, tensor-parallel over heads (2 heads per core).

Layout strategy (everything stays in "transposed" form on device so no fp32
transposes of activations are ever needed):
  - host feeds X^T [D, S]; device computes qkv^T = (X @ Wqkv_local)^T via
    matmul(lhsT=Wqkv_tile, rhs=X^T tile).
  - scores^T [t, s] per head via matmul(lhsT=K^T tile, rhs=Q^T chunk); the two
    heads of a core occupy partitions 0-63 / 64-127 and run row-packed on the
    PE array.
  - softmax without max-subtraction (scores/8 have sigma ~0.4; exp is safe in
    fp32), exp runs on ACT straight out of PSUM with the 1/8 scale folded in.
  - P^T @ V via matmul(lhsT=V_tile[t,d] with an appended ones column, rhs=P^T)
    -> numerator rows 0-63 and the softmax denominator on row 64 of PSUM.
  - denominator: reciprocal on row 64, broadcast back to partitions 0-63 with
    a K=1 matmul against a ones row, multiply.
  - y^T partial = Wo_local^T @ out^T accumulated over both heads; each core
    writes its full [D, S] fp32 partial; host sums the 8 partials, adds bo,
    transposes.
"""

import sys

sys.path.insert(0, "/opt/trn_rl_repo")

import functools
import numpy as np
import ml_dtypes

D = 1024
H = 16
HD = 64
NCORES = 8
HPC = H // NCORES  # heads per core = 2
P = 128
CH = 512  # s-chunk width
GROUP = 2  # score slots per exp group ([128, 1024] = 2 PSUM banks)

BF16 = ml_dtypes.bfloat16


def build_nc(S):
    import concourse.bacc as bacc
    import concourse.mybir as mybir
    from concourse import tile

    f32 = mybir.dt.float32
    f32r = mybir.dt.float32r
    bf16 = mybir.dt.bfloat16
    ADD = mybir.AluOpType.add
    EXP = mybir.ActivationFunctionType.Exp

    NCHUNK = S // CH
    NT = S // P  # number of 128-row t-tiles
    ND = D // P  # 8 d-tiles

    nc = bacc.Bacc("TRN2", target_bir_lowering=False, debug=False)

    xt_d = nc.dram_tensor("xt", [D, S], f32r, kind="ExternalInput")
    wqkv_d = nc.dram_tensor("wqkv", [D, 3 * HPC * HD], f32r, kind="ExternalInput")
    bqkv_d = nc.dram_tensor("bqkv", [3 * HPC * HD], f32, kind="ExternalInput")
    wo_d = nc.dram_tensor("wo", [HPC * HD, D], f32r, kind="ExternalInput")
    masks_d = nc.dram_tensor("masks", [P, 4, CH], bf16, kind="ExternalInput")
    ones_d = nc.dram_tensor("ones", [1, HD], f32r, kind="ExternalInput")
    yt_d = nc.dram_tensor("yt", [D, S], f32, kind="ExternalOutput")

    with tile.TileContext(nc) as tc:
        with (
            tc.tile_pool(name="consts", bufs=1) as consts,
            tc.tile_pool(name="xtp", bufs=18) as xtp,
            tc.tile_pool(name="ptp", bufs=8) as ptp,
            tc.tile_pool(name="otp", bufs=6) as otp,
            tc.tile_pool(name="rcp", bufs=4) as rcp,
            tc.tile_pool(name="vtp", bufs=2) as vtp,
            tc.tile_pool(name="ytp", bufs=8) as ytp,
            tc.tile_pool(name="scp", bufs=2, space="PSUM") as scp,
            tc.tile_pool(name="avp", bufs=2, space="PSUM") as avp,
            tc.tile_pool(name="qyp", bufs=2, space="PSUM") as qyp,
        ):
            # ---- persistent SBUF (wq first: first QKV matmuls need it) ----
            wq_sb = consts.tile([P, ND, 3 * HPC * HD], f32r)
            bq_sb = consts.tile([P, 3], f32)
            wo_sb = consts.tile([HPC * HD, D], f32r)
            masks_sb = consts.tile([P, 4, CH], bf16)
            ones_sb = consts.tile([P, HD], f32r)
            nc.sync.dma_start(bq_sb[:], bqkv_d[:].rearrange("(i p) -> p i", p=P))

            qt_sb = consts.tile([P, S], f32r)  # Q^T: h0 parts 0-63, h1 64-127
            kt_sb = consts.tile([P, S], f32r)
            vt_sb = consts.tile([P, S], f32)  # V^T
            # V-hat per head: [t-part, NT tiles, 72] (cols 0-63 = V, 64 = ones)
            vhat = [
                consts.tile([P, NT, 72], f32r, tag=f"vhat{h}", name=f"vhat{h}")
                for h in range(HPC)
            ]
            for h in range(HPC):
                nc.sync.dma_start(
                    vhat[h][:, :, 64:65],
                    ones_d[0:1, 0:NT].broadcast_to([P, NT]),
                )

            vt_r = vt_sb[:].rearrange("p (jt b f) -> p jt b f", b=4, f=32)

            def emit_qkv(j):
                """QKV^T projection for s-chunk j."""
                xts = []
                for d in range(ND):
                    if j == 0:  # interleave weight loads with the first x tiles
                        nc.sync.dma_start(
                            wq_sb[:, d, :], wqkv_d[d * P : (d + 1) * P, :]
                        )
                    xt_t = xtp.tile([P, CH], f32r, tag="xt", name="xt_t")
                    nc.sync.dma_start(
                        xt_t[:], xt_d[d * P : (d + 1) * P, j * CH : (j + 1) * CH]
                    )
                    xts.append(xt_t)
                for c in range(3):
                    ps = qyp.tile([P, CH], f32, tag="qy", name=f"qkvps{c}")
                    for d in range(ND):
                        nc.tensor.matmul(
                            ps[:],
                            wq_sb[:, d, c * P : (c + 1) * P],
                            xts[d][:],
                            start=(d == 0),
                            stop=(d == ND - 1),
                        )
                    dest = [qt_sb, kt_sb, vt_sb][c]
                    nc.vector.tensor_scalar(
                        out=dest[:, j * CH : (j + 1) * CH],
                        in0=ps[:],
                        scalar1=bq_sb[:, c : c + 1],
                        scalar2=None,
                        op0=ADD,
                    )

            emit_qkv(0)
            if NCHUNK > 1:
                emit_qkv(1)

            # cold-path constants: needed only ~10us in, after the first exp
            nc.sync.dma_start(masks_sb[:], masks_d[:])
            nc.sync.dma_start(ones_sb[64:65, :], ones_d[:])
            nc.sync.dma_start(wo_sb[:], wo_d[:])

            def emit_vhat(j):
                """V^T -> V-hat: f32 stream-transpose, then copy-round to f32r."""
                for h in range(HPC):
                    vtmp = vtp.tile([P, 4, 64], f32, tag="vtmp", name="vtmp")
                    for bj in range(2):
                        for bi in range(4):
                            nc.vector.transpose(
                                vtmp[
                                    32 * bi : 32 * bi + 32,
                                    :,
                                    32 * bj : 32 * bj + 32,
                                ],
                                vt_r[
                                    64 * h + 32 * bj : 64 * h + 32 * bj + 32,
                                    4 * j : 4 * j + 4,
                                    bi,
                                    :,
                                ],
                            )
                    nc.vector.tensor_copy(
                        vhat[h][:, 4 * j : 4 * j + 4, 0:64], vtmp[:]
                    )

            for j in range(NCHUNK):
                emit_vhat(j)
                # ---- attention for chunk j (groups pipelined by one) ----
                ntt = 4 * (j + 1)
                av = [
                    avp.tile([P, CH], f32, tag="av", name=f"av{h}")
                    for h in range(HPC)
                ]
                slots = [(tt, h) for tt in range(ntt) for h in range(HPC)]
                groups = [
                    slots[g0 : g0 + GROUP] for g0 in range(0, len(slots), GROUP)
                ]

                def soff(tt):
                    # diagonal tile k=tt-4j: columns s < 128k are fully masked
                    # (capped at 256 so fp32r matmuls keep free-dim >= 256)
                    o = (tt - 4 * j) * P if tt >= 4 * j else 0
                    return min(max(0, o), 2 * P)

                def flush(grp, sc):
                    pt = ptp.tile([P, GROUP * CH], f32r, tag="pt", name="pt")
                    L = len(grp)
                    o0 = soff(grp[0][0])
                    if all(soff(tt) == o0 for tt, _ in grp):
                        # one (possibly strided) exp over the valid columns
                        sc_v = sc[:].rearrange("p (g c) -> p g c", c=CH)
                        pt_v = pt[:].rearrange("p (g c) -> p g c", c=CH)
                        nc.scalar.activation(
                            pt_v[:, 0:L, o0:],
                            sc_v[:, 0:L, o0:],
                            EXP,
                            scale=0.125,
                        )
                    else:
                        for k, (tt, h) in enumerate(grp):
                            o = soff(tt)
                            nc.scalar.activation(
                                pt[:, k * CH + o : (k + 1) * CH],
                                sc[:, k * CH + o : (k + 1) * CH],
                                EXP,
                                scale=0.125,
                            )
                    if grp[0][0] >= 4 * j:  # diagonal: one masked mul for both heads
                        tt = grp[0][0]
                        o = soff(tt)
                        pt_v = pt[:].rearrange("p (g c) -> p g c", c=CH)
                        nc.vector.tensor_mul(
                            pt_v[:, 0:L, o:],
                            pt_v[:, 0:L, o:],
                            masks_sb[:, tt - 4 * j : tt - 4 * j + 1, o:].broadcast_to(
                                [P, L, CH - o]
                            ),
                        )
                    for k, (tt, h) in enumerate(grp):
                        o = soff(tt)
                        nc.tensor.matmul(
                            av[h][0:65, o:],
                            vhat[h][:, tt, 0:65],
                            pt[:, k * CH + o : (k + 1) * CH],
                            start=(tt == 0),
                            stop=(tt == ntt - 1),
                        )

                pending = None
                for grp in groups:
                    sc = scp.tile([P, GROUP * CH], f32, tag="sc", name="sc")
                    for k, (tt, h) in enumerate(grp):
                        o = soff(tt)
                        nc.tensor.matmul(
                            sc[:, k * CH + o : (k + 1) * CH],
                            kt_sb[64 * h : 64 * h + 64, tt * P : (tt + 1) * P],
                            qt_sb[
                                64 * h : 64 * h + 64, j * CH + o : (j + 1) * CH
                            ],
                            start=True,
                            stop=True,
                        )
                    if pending is not None:
                        flush(*pending)
                    pending = (grp, sc)
                if pending is not None:
                    flush(*pending)

                # ---- reciprocals + numerator copies (free the av tiles) ----
                rcs, nms = [], []
                for h in range(HPC):
                    rc = rcp.tile([P, CH], f32r, tag="rc", name="rc")
                    with nc.allow_low_precision("fp32r recip feeds fp22 matmul"):
                        nc.vector.reciprocal(rc[64:65, :], av[h][64:65, :])
                    nm = otp.tile([HD, CH], f32, tag="nm", name="nm")
                    nc.vector.tensor_copy(nm[:], av[h][0:64, :])
                    rcs.append(rc)
                    nms.append(nm)

                # ---- chunk j+2's QKV keeps PE busy during the div chain ----
                if j + 2 < NCHUNK:
                    emit_qkv(j + 2)

                # ---- denominator broadcast + divide ----
                ot = otp.tile([P, CH], f32r, tag="ot", name="ot")
                for h in range(HPC):
                    bc = qyp.tile([HD, CH], f32, tag="qy", name="bc")
                    nc.tensor.matmul(
                        bc[:],
                        ones_sb[64:65, 0:HD],
                        rcs[h][64:65, :],
                        start=True,
                        stop=True,
                    )
                    nc.vector.tensor_mul(
                        ot[64 * h : 64 * h + 64, :], nms[h][:], bc[:]
                    )

                # ---- output projection for chunk j ----
                for e in range(ND):
                    yt_ps = qyp.tile([P, CH], f32, tag="qy", name="ytps")
                    nc.tensor.matmul(
                        yt_ps[:],
                        wo_sb[:, e * P : (e + 1) * P],
                        ot[:],
                        start=True,
                        stop=True,
                    )
                    yt_sb = ytp.tile([P, CH], f32, tag="yt", name="ytsb")
                    nc.vector.tensor_copy(yt_sb[:], yt_ps[:])
                    nc.sync.dma_start(
                        yt_d[e * P : (e + 1) * P, j * CH : (j + 1) * CH],
                        yt_sb[:],
                    )

    return nc


@functools.lru_cache(maxsize=2)
def _get_nc(S):
    nc = build_nc(S)
    nc.compile()
    return nc


def make_in_maps(input, Wqkv, bqkv, Wo, S):
    """Host-side shard prep. input [1,S,D] (or [S,D]); returns per-core dicts."""
    x = np.asarray(input, dtype=np.float32).reshape(S, D)
    xt = np.ascontiguousarray(x.T)
    Wqkv = np.asarray(Wqkv, dtype=np.float32)
    bqkv = np.asarray(bqkv, dtype=np.float32)
    Wo = np.asarray(Wo, dtype=np.float32)

    # causal masks for the 4 diagonal 128-blocks of a 512 chunk
    pp = np.arange(P)[:, None]
    ff = np.arange(CH)[None, :]
    masks = np.stack(
        [(ff >= pp + P * k).astype(BF16) for k in range(4)], axis=1
    )  # [128, 4, 512]
    masks = np.ascontiguousarray(masks)

    Wq, Wk, Wv = Wqkv[:, 0:D], Wqkv[:, D : 2 * D], Wqkv[:, 2 * D : 3 * D]
    bq, bk, bv = bqkv[0:D], bqkv[D : 2 * D], bqkv[2 * D : 3 * D]

    in_maps = []
    for c in range(NCORES):
        hs = [c * HPC + i for i in range(HPC)]
        cols = lambda W: np.concatenate(
            [W[:, h * HD : (h + 1) * HD] for h in hs], axis=1
        )
        colsb = lambda b: np.concatenate(
            [b[h * HD : (h + 1) * HD] for h in hs], axis=0
        )
        wqkv_l = np.ascontiguousarray(
            np.concatenate([cols(Wq), cols(Wk), cols(Wv)], axis=1)
        )
        bqkv_l = np.ascontiguousarray(
            np.concatenate([colsb(bq), colsb(bk), colsb(bv)], axis=0)
        )
        wo_l = np.ascontiguousarray(Wo[hs[0] * HD : hs[0] * HD + HPC * HD, :])
        in_maps.append(
            {
                "xt": xt,
                "wqkv": wqkv_l,
                "bqkv": bqkv_l,
                "wo": wo_l,
                "masks": masks,
                "ones": np.ones((1, HD), dtype=np.float32),
            }
        )
    return in_maps


def kernel(input, Wqkv, bqkv, Wo, bo):
    from concourse.bass_utils import run_bass_kernel_spmd

    S = np.asarray(input).reshape(-1, D).shape[0]
    nc = _get_nc(S)
    in_maps = make_in_maps(input, Wqkv, bqkv, Wo, S)
    res = None
    last_exc = None
    for _attempt in range(3):  # transient NRT/device errors: retry
        try:
            res = run_bass_kernel_spmd(nc, in_maps, core_ids=list(range(NCORES)))
            break
        except Exception as e:  # noqa: BLE001
            last_exc = e
    if res is None:
        raise last_exc
    yt = res.results[0]["yt"].copy()
    for r in res.results[1:]:
        yt += r["yt"]
    y = yt.T + np.asarray(bo, dtype=np.float32)[None, :]
    return np.ascontiguousarray(y, dtype=np.float32).reshape(1, S, D)



# revision 36
# speedup vs baseline: 1.0910x; 1.0910x over previous
"""Multi-head causal self-attention (B=1, S=4096, D=1024, H=16) on 8 TRN2
NeuronCores, tensor-parallel over heads (2 heads per core).

v3 design (vs the fp32r baseline):
  - bf16 operands on the PE except the score matmuls, which run fp8e4m3 +
    DoubleRow (0.5 cyc/col, contraction 64 packed as [32 partitions x 2]).
  - V is produced directly in [s-part, d] orientation by swapping matmul
    operands (lhsT = X^T tile), eliminating all DVE stream-transposes.
  - k-bias dropped (softmax-invariant); v-bias folded into a host constant
    (bv @ Wo added with bo); only the q-bias is applied on device.
  - Causal masking multiplies only the 128x128 diagonal triangle per tile.
  - Softmax divide reads AV PSUM directly; denominator reciprocal feeds a
    K=1 broadcast matmul.
  - Output-projection PSUM pairs are copied to SBUF on the idle Pool engine
    and DMA'd out as fp32 partials summed on host.
  - Scheduling: per-chunk tail work (divide, oproj, next chunk's QKV) is
    split into small fillers interleaved between score/exp groups of the
    following chunk so the ACT engine (exp) never starves; the first three
    AV matmuls of each chunk are deferred so they don't head-of-line block
    the PE queue while the previous chunk's divide chain frees the AV banks.
"""

import sys

sys.path.insert(0, "/opt/trn_rl_repo")

import functools
import numpy as np
import ml_dtypes

D = 1024
H = 16
HD = 64
NCORES = 8
HPC = H // NCORES  # heads per core = 2
P = 128
CH = 512  # s-chunk width
GROUP = 2  # (tt, h0), (tt, h1) share an exp group

BF16 = ml_dtypes.bfloat16


def build_nc(S):
    import concourse.bacc as bacc
    import concourse.mybir as mybir
    from concourse import tile
    from collections import deque

    f32 = mybir.dt.float32
    f32r = mybir.dt.float32r
    bf16 = mybir.dt.bfloat16
    f8e4 = mybir.dt.float8e4
    DR = mybir.MatmulPerfMode.DoubleRow
    ADD = mybir.AluOpType.add
    EXP = mybir.ActivationFunctionType.Exp

    NCHUNK = S // CH
    NT = S // P  # number of 128-row t-tiles
    ND = D // P  # 8 d-tiles

    nc = bacc.Bacc("TRN2", target_bir_lowering=False, debug=False)

    xt_d = nc.dram_tensor("xt", [D, S], bf16, kind="ExternalInput")
    wqk_d = nc.dram_tensor("wqk", [P, ND, 2 * P], bf16, kind="ExternalInput")
    wv_d = nc.dram_tensor("wv", [P, ND, P], bf16, kind="ExternalInput")
    wo_d = nc.dram_tensor("wo", [P, D], bf16, kind="ExternalInput")
    bq_d = nc.dram_tensor("bq", [P, 1], f32, kind="ExternalInput")
    tri_d = nc.dram_tensor("tri", [P, P], bf16, kind="ExternalInput")
    onesb_d = nc.dram_tensor("onesb", [1, NT], bf16, kind="ExternalInput")
    onesf_d = nc.dram_tensor("onesf", [1, HD], f32r, kind="ExternalInput")
    yt_d = nc.dram_tensor("yt", [D, S], f32, kind="ExternalOutput")

    with tile.TileContext(nc) as tc:
        with (
            tc.tile_pool(name="consts", bufs=1) as consts,
            tc.tile_pool(name="xtp", bufs=3) as xtp,
            tc.tile_pool(name="qk8p", bufs=4) as qk8p,
            tc.tile_pool(name="qdp", bufs=2) as qdp,
            tc.tile_pool(name="ptp", bufs=7) as ptp,
            tc.tile_pool(name="otp", bufs=2) as otp,
            tc.tile_pool(name="rcp", bufs=4) as rcp,
            tc.tile_pool(name="bcp", bufs=4) as bcp,
            tc.tile_pool(name="ytp", bufs=3) as ytp,
            tc.tile_pool(name="scp", bufs=2, space="PSUM") as scp,
            tc.tile_pool(name="avp", bufs=2, space="PSUM") as avp,
            tc.tile_pool(name="qyp", bufs=2, space="PSUM") as qyp,
        ):
            # ---- persistent SBUF ----
            wqk_sb = consts.tile([P, ND, 2 * P], bf16)
            wv_sb = consts.tile([P, ND, P], bf16)
            wo_sb = consts.tile([P, D], bf16)
            bq_sb = consts.tile([P, 1], f32)
            tri_sb = consts.tile([P, P], bf16)
            onesf_sb = consts.tile([1, HD], f32r)

            # K in DoubleRow layout [32*h + d%32, d//32, t], fp8
            kd_sb = consts.tile([32 * HPC, 2, S], f8e4, name="kd")
            # V-hat: [t-part, NT, 2*(64+1)] cols: h0 v | ones | h1 v | ones
            vhat = consts.tile([P, NT, 2 * (HD + 1)], bf16, name="vhat")
            nc.gpsimd.memset(vhat[:, :, HD : HD + 1], 1.0)
            nc.gpsimd.memset(vhat[:, :, 2 * HD + 1 : 2 * HD + 2], 1.0)

            filler = deque()  # (key, fn) PE work bundles between groups

            def pop_filler(n=1):
                for _ in range(n):
                    if filler:
                        filler.popleft()[1]()

            def drain_filler(key):
                """Run all queued fillers up to and including the last one
                tagged `key` (chunk-j qkv work chunk j depends on)."""
                while any(k == key for k, _ in filler):
                    pop_filler()

            def load_x(j):
                """One DMA for the whole [128, ND, CH] X^T chunk."""
                xt_t = xtp.tile([P, ND, CH], bf16, tag="xt", name="xt_t")
                nc.sync.dma_start(
                    xt_t[:],
                    xt_d[:, j * CH : (j + 1) * CH].rearrange(
                        "(d p) s -> p d s", p=P
                    ),
                )
                return xt_t

            def qk_mm(xt_t, c, ps, dlo, dhi):
                for d in range(dlo, dhi):
                    nc.tensor.matmul(
                        ps[:],
                        wqk_sb[:, d, c * P : (c + 1) * P],
                        xt_t[:, d, :],
                        start=(d == 0),
                        stop=(d == ND - 1),
                    )

            def qk_fin(j, c, ps, qd_dst=None):
                """fp8 conversion (+ q bias) and DR-layout remap."""
                q8 = qk8p.tile([P, CH], f8e4, tag="qk8", name=f"qk8{c}")
                if c == 0:
                    nc.vector.tensor_scalar(
                        out=q8[:], in0=ps[:], scalar1=bq_sb[:, 0:1],
                        scalar2=None, op0=ADD,
                    )
                else:
                    nc.vector.tensor_copy(q8[:], ps[:])
                if c == 0:
                    dcol = qd_dst[:, :, 0:CH]
                else:
                    dcol = kd_sb[:, :, j * CH : (j + 1) * CH]
                # 4 plain rectangles: a multi-partition-dim source AP
                # lowers to garbage on HW
                for h in range(HPC):
                    for i in range(2):
                        nc.sync.dma_start(
                            dcol[h * 32 : h * 32 + 32, i, :],
                            q8[h * HD + i * 32 : h * HD + i * 32 + 32, :],
                        )

            def emit_qk(j, xt_t, c, qd_dst=None):
                ps = qyp.tile([P, CH], f32, tag="qy", name=f"qkps{c}")
                qk_mm(xt_t, c, ps, 0, ND)
                qk_fin(j, c, ps, qd_dst)

            def emit_v(j, xt_t, st):
                """V in direct [s, d] orientation for one 128-row s-tile."""
                vps = qyp.tile([P, P], f32, tag="qy", name="vps")
                for d in range(ND):
                    nc.tensor.matmul(
                        vps[:],
                        xt_t[:, d, st * P : (st + 1) * P],
                        wv_sb[:, d, :],
                        start=(d == 0),
                        stop=(d == ND - 1),
                    )
                vh = vhat[:].rearrange("p t (h c) -> p t h c", h=2)
                nc.vector.tensor_copy(
                    vh[:, 4 * j + st, :, 0:HD],
                    vps[:].rearrange("p (h c) -> p h c", h=2),
                )

            def queue_qkv_fillers(j, xt_t, dst):
                """Fine-grained (key, fn) fillers (~0.5-0.9us PE work each)."""
                out = []
                for c in range(2):
                    ps = qyp.tile([P, CH], f32, tag="qy", name=f"qkps{c}")
                    out.append(
                        (("qk", j), lambda c=c, ps=ps: qk_mm(xt_t, c, ps, 0, 4))
                    )
                    out.append(
                        (
                            ("qk", j),
                            lambda c=c, ps=ps: (
                                qk_mm(xt_t, c, ps, 4, ND),
                                qk_fin(j, c, ps, dst),
                            ),
                        )
                    )
                for st in range(4):
                    out.append((("v", j), lambda st=st: emit_v(j, xt_t, st)))
                return out

            def queue_tail(j, av):
                """divide + output projection of chunk j as fillers.

                Caller interleaves these with the next chunk's qkv fillers.
                """
                rcs, nms = [], []
                for h in range(HPC):
                    rc = rcp.tile([1, CH], f32r, tag="rc", name="rc")
                    with nc.allow_low_precision("recip feeds fp22 matmul"):
                        nc.vector.reciprocal(rc[:], av[h][HD : HD + 1, :])
                    rcs.append(rc)
                    # numerator copy also frees the av bank for the next chunk
                    nm = bcp.tile([HD, CH], bf16, tag="nm", name="nm")
                    nc.vector.tensor_copy(nm[:], av[h][0:HD, :])
                    nms.append(nm)
                ot = otp.tile([P, CH], bf16, tag="ot", name="ot")

                def bc_ot(h):
                    bc = qyp.tile([P, CH], f32, tag="qy", name="bc")
                    nc.tensor.matmul(
                        bc[0:HD, :], onesf_sb[0:1, :], rcs[h][:],
                        start=True, stop=True,
                    )
                    nc.vector.tensor_mul(
                        ot[h * HD : (h + 1) * HD, :],
                        nms[h][:],
                        bc[0:HD, :],
                    )

                last = j == NCHUNK - 1

                def oproj(e2):
                    yt_sb = ytp.tile([P, 2, CH], f32, tag="yt", name="ytsb")
                    for k in range(2):
                        yt_ps = qyp.tile([P, CH], f32, tag="qy", name="ytps")
                        nc.tensor.matmul(
                            yt_ps[:],
                            wo_sb[:, (2 * e2 + k) * P : (2 * e2 + k + 1) * P],
                            ot[:],
                            start=True,
                            stop=True,
                        )
                        # Pool can't read PSUM (walrus verifier), so the
                        # output copies run on DVE
                        nc.vector.tensor_copy(yt_sb[:, k, :], yt_ps[:])
                    nc.sync.dma_start(
                        yt_d[:, j * CH : (j + 1) * CH].rearrange(
                            "(e p) s -> p e s", p=P
                        )[:, 2 * e2 : 2 * e2 + 2, :],
                        yt_sb[:],
                    )

                return [(None, lambda: bc_ot(0)), (None, lambda: bc_ot(1))] + [
                    (None, lambda e2=e2: oproj(e2)) for e2 in range(ND // 2)
                ]

            # ---- startup ----
            # x chunk 0 first (its transfer overlaps the weight DMAs), then
            # warm the PE with dummy K=1 matmuls so the p-state ramp finishes
            # before the real QKV work arrives.
            xt0 = load_x(0)
            nc.sync.dma_start(wqk_sb[:], wqk_d[:])
            nc.sync.dma_start(wv_sb[:], wv_d[:])
            xt1 = load_x(1) if NCHUNK > 1 else None
            nc.sync.dma_start(bq_sb[:], bq_d[:])
            warm = consts.tile([1, CH], bf16)
            nc.gpsimd.memset(warm[:], 1.0)
            for w in range(18):
                wps = qyp.tile([P, CH], f32, tag="qy", name="warm")
                nc.tensor.matmul(
                    wps[:], warm[0:1, 0:P], warm[:], start=True, stop=True
                )

            qds = [None, None]
            qds[0] = qdp.tile([32 * HPC, 2, CH], f8e4, tag="qd", name="qd")
            emit_qk(0, xt0, 0, qds[0])
            emit_qk(0, xt0, 1)
            nc.sync.dma_start(tri_sb[:], tri_d[:])
            nc.sync.dma_start(onesf_sb[:], onesf_d[:])
            nc.sync.dma_start(wo_sb[:], wo_d[:])
            for st in range(4):
                filler.append((("v", 0), lambda st=st: emit_v(0, xt0, st)))
            if NCHUNK > 1:
                qds[1] = qdp.tile([32 * HPC, 2, CH], f8e4, tag="qd", name="qd")
                filler.extend(queue_qkv_fillers(1, xt1, qds[1]))

            for j in range(NCHUNK):
                drain_filler(("qk", j))  # chunk j's q/k must be in place
                qd = qds[j % 2]
                ntt = 4 * (j + 1)
                av = [
                    avp.tile([P, CH], f32, tag="av", name=f"av{h}")
                    for h in range(HPC)
                ]

                def soff(tt):
                    return (tt - 4 * j) * P if tt >= 4 * j else 0

                def issue_av(tt, pt, o):
                    for h in range(HPC):
                        nc.tensor.matmul(
                            av[h][0 : HD + 1, o:],
                            vhat[:, tt, h * (HD + 1) : (h + 1) * (HD + 1)],
                            pt[:, h, o:],
                            start=(tt == 0),
                            stop=(tt == ntt - 1),
                        )

                def flush_exp(tt, sc):
                    o = soff(tt)
                    pt = ptp.tile([P, GROUP, CH], bf16, tag="pt", name="pt")
                    sc_v = sc[:].rearrange("p (g c) -> p g c", c=CH)
                    nc.scalar.activation(
                        pt[:, :, o:], sc_v[:, :, o:], EXP, scale=0.125
                    )
                    if tt >= 4 * j:  # diagonal: mask the 128-wide triangle
                        nc.vector.tensor_mul(
                            pt[:, :, o : o + P],
                            pt[:, :, o : o + P],
                            tri_sb[:].rearrange("p (g c) -> p g c", g=1).broadcast_to(
                                [P, GROUP, P]
                            ),
                        )
                    return tt, pt, o

                # first 3 AVs of a chunk are deferred (j>0) so the PE queue
                # isn't blocked while the previous divide frees the av banks
                defer = 3 if j > 0 else 0
                av_backlog = []
                pending = None
                for g, tt in enumerate(range(ntt)):
                    if tt == 4 * j:  # diagonal flushes need chunk j's V
                        drain_filler(("v", j))
                    o = soff(tt)
                    sc = scp.tile([P, GROUP * CH], f32, tag="sc", name="sc")
                    for h in range(HPC):
                        nc.tensor.matmul(
                            sc[:, h * CH + o : (h + 1) * CH],
                            kd_sb[h * 32 : h * 32 + 32, :, tt * P : (tt + 1) * P],
                            qd[h * 32 : h * 32 + 32, :, o:CH],
                            start=True,
                            stop=True,
                            perf_mode=DR,
                        )
                    if pending is not None:
                        fl = flush_exp(*pending)
                        if len(av_backlog) < defer and g <= defer:
                            av_backlog.append(fl)
                        else:
                            if av_backlog:
                                for b in av_backlog:
                                    issue_av(*b)
                                av_backlog = []
                            issue_av(*fl)
                    pending = (tt, sc)
                    pop_filler(1 + (len(filler) > 10))
                if pending is not None:
                    fl = flush_exp(*pending)
                    for b in av_backlog:
                        issue_av(*b)
                    issue_av(*fl)

                # ---- boundary work: interleave tail of j with qkv of j+2 ----
                tail = queue_tail(j, av)
                if j + 2 < NCHUNK:
                    jn = j + 2
                    xt_t = load_x(jn)
                    dst = qdp.tile([32 * HPC, 2, CH], f8e4, tag="qd", name="qd")
                    qkv = queue_qkv_fillers(jn, xt_t, dst)
                    # order: a qk piece first (covers the recip latency), the
                    # divide, the rest of qkv, then the output projections
                    filler.append(qkv[0])
                    filler.append(tail[0])  # bc_ot(0)
                    filler.append(tail[1])  # bc_ot(1)
                    filler.extend(qkv[1:])
                    filler.extend(tail[2:])  # oproj pairs
                    qds[j % 2] = dst
                else:
                    filler.extend(tail)

            pop_filler(len(filler))

    return nc


@functools.lru_cache(maxsize=2)
def _get_nc(S):
    nc = build_nc(S)
    nc.compile()
    return nc


def make_in_maps(input, Wqkv, bqkv, Wo, S):
    """Host-side shard prep. Returns per-core input dicts."""
    x = np.asarray(input, dtype=np.float32).reshape(S, D)
    xt = np.ascontiguousarray(x.T).astype(BF16)
    Wqkv = np.asarray(Wqkv, dtype=np.float32)
    bqkv = np.asarray(bqkv, dtype=np.float32)
    Wo = np.asarray(Wo, dtype=np.float32)

    NT = S // P
    ND = D // P
    Wq, Wk, Wv = Wqkv[:, 0:D], Wqkv[:, D : 2 * D], Wqkv[:, 2 * D : 3 * D]
    bq = bqkv[0:D]

    tri = np.triu(np.ones((P, P), dtype=np.float32)).astype(BF16)  # s >= t

    in_maps = []
    for c in range(NCORES):
        hs = [c * HPC + i for i in range(HPC)]
        cols = lambda W: np.concatenate(
            [W[:, h * HD : (h + 1) * HD] for h in hs], axis=1
        )  # [D, 128] in (h, d) order
        wq_l, wk_l, wv_l = cols(Wq), cols(Wk), cols(Wv)
        wqk_l = np.concatenate([wq_l, wk_l], axis=1)  # [D, 256]
        wqk_l = np.ascontiguousarray(
            wqk_l.reshape(ND, P, 2 * P).transpose(1, 0, 2)
        ).astype(BF16)  # [128, ND, 256] with [p, d, c] = W[d*128+p, c]
        wv_l3 = np.ascontiguousarray(
            wv_l.reshape(ND, P, P).transpose(1, 0, 2)
        ).astype(BF16)
        wo_l = np.ascontiguousarray(Wo[hs[0] * HD : hs[0] * HD + HPC * HD, :]).astype(
            BF16
        )
        bq_l = np.concatenate([bq[h * HD : (h + 1) * HD] for h in hs])[:, None]
        in_maps.append(
            {
                "xt": xt,
                "wqk": wqk_l,
                "wv": wv_l3,
                "wo": wo_l,
                "bq": np.ascontiguousarray(bq_l, dtype=np.float32),
                "tri": tri,
                "onesb": np.ones((1, NT), dtype=BF16),
                "onesf": np.ones((1, HD), dtype=np.float32),
            }
        )
    return in_maps


def kernel(input, Wqkv, bqkv, Wo, bo):
    from concourse.bass_utils import run_bass_kernel_spmd

    S = np.asarray(input).reshape(-1, D).shape[0]
    nc = _get_nc(S)
    in_maps = make_in_maps(input, Wqkv, bqkv, Wo, S)
    res = None
    last_exc = None
    for _attempt in range(3):  # transient NRT/device errors: retry
        try:
            res = run_bass_kernel_spmd(nc, in_maps, core_ids=list(range(NCORES)))
            break
        except Exception as e:  # noqa: BLE001
            last_exc = e
    if res is None:
        raise last_exc
    yt = res.results[0]["yt"].copy()
    for r in res.results[1:]:
        yt += r["yt"]
    bqkv = np.asarray(bqkv, dtype=np.float32)
    bv = bqkv[2 * D : 3 * D]
    Wo_f = np.asarray(Wo, dtype=np.float32)
    y = yt.T + (np.asarray(bo, dtype=np.float32) + bv @ Wo_f)[None, :]
    return np.ascontiguousarray(y, dtype=np.float32).reshape(1, S, D)


# revision 45
# speedup vs baseline: 1.0935x; 1.0023x over previous
"""Multi-head causal self-attention (B=1, S=4096, D=1024, H=16) on 8 TRN2
NeuronCores, tensor-parallel over heads (2 heads per core).

v3 design (vs the fp32r baseline):
  - bf16 operands on the PE except the score matmuls, which run fp8e4m3 +
    DoubleRow (0.5 cyc/col, contraction 64 packed as [32 partitions x 2]).
  - V is produced directly in [s-part, d] orientation by swapping matmul
    operands (lhsT = X^T tile), eliminating all DVE stream-transposes.
  - k-bias dropped (softmax-invariant); v-bias folded into a host constant
    (bv @ Wo added with bo); only the q-bias is applied on device.
  - Causal masking multiplies only the 128x128 diagonal triangle per tile.
  - Softmax divide reads AV PSUM directly; denominator reciprocal feeds a
    K=1 broadcast matmul.
  - Output-projection PSUM pairs are copied to SBUF on the idle Pool engine
    and DMA'd out as fp32 partials summed on host.
  - Scheduling: per-chunk tail work (divide, oproj, next chunk's QKV) is
    split into small fillers interleaved between score/exp groups of the
    following chunk so the ACT engine (exp) never starves; the first three
    AV matmuls of each chunk are deferred so they don't head-of-line block
    the PE queue while the previous chunk's divide chain frees the AV banks.
"""

import sys

sys.path.insert(0, "/opt/trn_rl_repo")

import functools
import numpy as np
import ml_dtypes

D = 1024
H = 16
HD = 64
NCORES = 8
HPC = H // NCORES  # heads per core = 2
P = 128
CH = 512  # s-chunk width
GROUP = 2  # (tt, h0), (tt, h1) share an exp group

BF16 = ml_dtypes.bfloat16


def build_nc(S):
    import concourse.bacc as bacc
    import concourse.mybir as mybir
    from concourse import tile
    from collections import deque

    f32 = mybir.dt.float32
    f32r = mybir.dt.float32r
    bf16 = mybir.dt.bfloat16
    f8e4 = mybir.dt.float8e4
    DR = mybir.MatmulPerfMode.DoubleRow
    ADD = mybir.AluOpType.add
    MUL = mybir.AluOpType.mult
    EXP = mybir.ActivationFunctionType.Exp

    NCHUNK = S // CH
    NT = S // P  # number of 128-row t-tiles
    ND = D // P  # 8 d-tiles

    nc = bacc.Bacc("TRN2", target_bir_lowering=False, debug=False)

    NDP = ND // 2  # d-tile pairs (DoubleRow contraction 256)
    # X^T and W as fp8 + fp8 residual; W pre-scaled by 32 on host, the 1/32
    # is applied when PSUM is read back
    x8_d = nc.dram_tensor("x8", [P, NDP, 2, S], f8e4, kind="ExternalInput")
    xr8_d = nc.dram_tensor("xr8", [P, NDP, 2, S], f8e4, kind="ExternalInput")
    wqk_d = nc.dram_tensor("wqk", [P, NDP, 2, 2 * P], f8e4, kind="ExternalInput")
    wqkr_d = nc.dram_tensor("wqkr", [P, NDP, 2, 2 * P], f8e4, kind="ExternalInput")
    wv_d = nc.dram_tensor("wv", [P, NDP, 2, P], f8e4, kind="ExternalInput")
    wvr_d = nc.dram_tensor("wvr", [P, NDP, 2, P], f8e4, kind="ExternalInput")
    wo_d = nc.dram_tensor("wo", [P, D], bf16, kind="ExternalInput")
    bq_d = nc.dram_tensor("bq", [P, 1], f32, kind="ExternalInput")
    tri_d = nc.dram_tensor("tri", [P, P], bf16, kind="ExternalInput")
    onesb_d = nc.dram_tensor("onesb", [1, NT], bf16, kind="ExternalInput")
    onesf_d = nc.dram_tensor("onesf", [1, HD], f32r, kind="ExternalInput")
    yt_d = nc.dram_tensor("yt", [D, S], f32, kind="ExternalOutput")

    with tile.TileContext(nc) as tc:
        with (
            tc.tile_pool(name="consts", bufs=1) as consts,
            tc.tile_pool(name="xtp", bufs=3) as xtp,
            tc.tile_pool(name="qk8p", bufs=4) as qk8p,
            tc.tile_pool(name="qdp", bufs=2) as qdp,
            tc.tile_pool(name="ptp", bufs=7) as ptp,
            tc.tile_pool(name="otp", bufs=2) as otp,
            tc.tile_pool(name="rcp", bufs=4) as rcp,
            tc.tile_pool(name="bcp", bufs=4) as bcp,
            tc.tile_pool(name="ytp", bufs=3) as ytp,
            tc.tile_pool(name="scp", bufs=2, space="PSUM") as scp,
            tc.tile_pool(name="avp", bufs=2, space="PSUM") as avp,
            tc.tile_pool(name="qyp", bufs=2, space="PSUM") as qyp,
        ):
            # ---- persistent SBUF ----
            wqk_sb = consts.tile([P, NDP, 2, 2 * P], f8e4)
            wqkr_sb = consts.tile([P, NDP, 2, 2 * P], f8e4)
            wv_sb = consts.tile([P, NDP, 2, P], f8e4)
            wvr_sb = consts.tile([P, NDP, 2, P], f8e4)
            wo_sb = consts.tile([P, D], bf16)
            bq_sb = consts.tile([P, 1], f32)
            tri_sb = consts.tile([P, P], bf16)
            onesf_sb = consts.tile([1, HD], f32r)

            # K in DoubleRow layout [32*h + d%32, d//32, t], fp8
            kd_sb = consts.tile([32 * HPC, 2, S], f8e4, name="kd")
            # V-hat: [t-part, NT, 2*(64+1)] cols: h0 v | ones | h1 v | ones
            vhat = consts.tile([P, NT, 2 * (HD + 1)], bf16, name="vhat")
            nc.gpsimd.memset(vhat[:, :, HD : HD + 1], 1.0)
            nc.gpsimd.memset(vhat[:, :, 2 * HD + 1 : 2 * HD + 2], 1.0)

            filler = deque()  # (key, fn) PE work bundles between groups

            def pop_filler(n=1):
                for _ in range(n):
                    if filler:
                        filler.popleft()[1]()

            def drain_filler(key):
                """Run all queued fillers up to and including the last one
                tagged `key` (chunk-j qkv work chunk j depends on)."""
                while any(k == key for k, _ in filler):
                    pop_filler()

            def load_x(j):
                """Two DMAs for the chunk's fp8 X^T + residual."""
                x8_t = xtp.tile([P, NDP, 2, CH], f8e4, tag="xt", name="x8_t")
                xr8_t = xtp.tile([P, NDP, 2, CH], f8e4, tag="xt", name="xr8_t")
                nc.sync.dma_start(x8_t[:], x8_d[:, :, :, j * CH : (j + 1) * CH])
                nc.sync.dma_start(xr8_t[:], xr8_d[:, :, :, j * CH : (j + 1) * CH])
                return (x8_t, xr8_t)

            def qk_mm(xt_t, c, ps, dlo, dhi):
                """3-chain compensated fp8: x8@w8 + xr8@w8 + x8@wr8."""
                x8_t, xr8_t = xt_t
                chains = [(wqk_sb, x8_t), (wqk_sb, xr8_t), (wqkr_sb, x8_t)]
                for d in range(dlo, dhi):
                    for ci, (w, x) in enumerate(chains):
                        nc.tensor.matmul(
                            ps[:],
                            w[:, d, :, c * P : (c + 1) * P],
                            x[:, d, :, :],
                            start=(d == 0 and ci == 0),
                            stop=(d == NDP - 1 and ci == len(chains) - 1),
                            perf_mode=DR,
                        )

            def qk_fin(j, c, ps, qd_dst=None):
                """fp8 conversion (1/32 weight scale, + q bias) and remap."""
                q8 = qk8p.tile([P, CH], f8e4, tag="qk8", name=f"qk8{c}")
                if c == 0:
                    nc.vector.tensor_scalar(
                        out=q8[:], in0=ps[:], scalar1=1.0 / 32, scalar2=bq_sb[:, 0:1],
                        op0=MUL, op1=ADD,
                    )
                else:
                    nc.vector.tensor_scalar(
                        out=q8[:], in0=ps[:], scalar1=1.0 / 32, scalar2=None,
                        op0=MUL,
                    )
                if c == 0:
                    dcol = qd_dst[:, :, 0:CH]
                else:
                    dcol = kd_sb[:, :, j * CH : (j + 1) * CH]
                # 4 plain rectangles: a multi-partition-dim source AP
                # lowers to garbage on HW
                for h in range(HPC):
                    for i in range(2):
                        nc.sync.dma_start(
                            dcol[h * 32 : h * 32 + 32, i, :],
                            q8[h * HD + i * 32 : h * HD + i * 32 + 32, :],
                        )

            def emit_qk(j, xt_t, c, qd_dst=None):
                ps = qyp.tile([P, CH], f32, tag="qy", name=f"qkps{c}")
                qk_mm(xt_t, c, ps, 0, NDP)
                qk_fin(j, c, ps, qd_dst)

            def emit_v(j, xt_t, st):
                """V in direct [s, d] orientation for one 128-row s-tile."""
                x8_t, xr8_t = xt_t
                vps = qyp.tile([P, P], f32, tag="qy", name="vps")
                chains = [(x8_t, wv_sb), (xr8_t, wv_sb), (x8_t, wvr_sb)]
                for d in range(NDP):
                    for ci, (x, w) in enumerate(chains):
                        nc.tensor.matmul(
                            vps[:],
                            x[:, d, :, st * P : (st + 1) * P],
                            w[:, d, :, :],
                            start=(d == 0 and ci == 0),
                            stop=(d == NDP - 1 and ci == len(chains) - 1),
                            perf_mode=DR,
                        )
                vh = vhat[:].rearrange("p t (h c) -> p t h c", h=2)
                nc.vector.tensor_scalar(
                    out=vh[:, 4 * j + st, :, 0:HD],
                    in0=vps[:].rearrange("p (h c) -> p h c", h=2),
                    scalar1=1.0 / 32,
                    scalar2=None,
                    op0=MUL,
                )

            def queue_qkv_fillers(j, xt_t, dst):
                """Fine-grained (key, fn) fillers (~0.5-0.9us PE work each)."""
                out = []
                for c in range(2):
                    ps = qyp.tile([P, CH], f32, tag="qy", name=f"qkps{c}")
                    out.append(
                        (("qk", j), lambda c=c, ps=ps: qk_mm(xt_t, c, ps, 0, 2))
                    )
                    out.append(
                        (
                            ("qk", j),
                            lambda c=c, ps=ps: (
                                qk_mm(xt_t, c, ps, 2, NDP),
                                qk_fin(j, c, ps, dst),
                            ),
                        )
                    )
                for st in range(4):
                    out.append((("v", j), lambda st=st: emit_v(j, xt_t, st)))
                return out

            def queue_tail(j, av):
                """divide + output projection of chunk j as fillers.

                Caller interleaves these with the next chunk's qkv fillers.
                """
                rcs, nms = [], []
                for h in range(HPC):
                    rc = rcp.tile([1, CH], f32r, tag="rc", name="rc")
                    with nc.allow_low_precision("recip feeds fp22 matmul"):
                        nc.vector.reciprocal(rc[:], av[h][HD : HD + 1, :])
                    rcs.append(rc)
                    # numerator copy also frees the av bank for the next chunk
                    nm = bcp.tile([HD, CH], bf16, tag="nm", name="nm")
                    nc.vector.tensor_copy(nm[:], av[h][0:HD, :])
                    nms.append(nm)
                ot = otp.tile([P, CH], bf16, tag="ot", name="ot")

                def bc_ot(h):
                    bc = qyp.tile([P, CH], f32, tag="qy", name="bc")
                    nc.tensor.matmul(
                        bc[0:HD, :], onesf_sb[0:1, :], rcs[h][:],
                        start=True, stop=True,
                    )
                    nc.vector.tensor_mul(
                        ot[h * HD : (h + 1) * HD, :],
                        nms[h][:],
                        bc[0:HD, :],
                    )

                last = j == NCHUNK - 1

                def oproj(e2):
                    yt_sb = ytp.tile([P, 2, CH], f32, tag="yt", name="ytsb")
                    for k in range(2):
                        yt_ps = qyp.tile([P, CH], f32, tag="qy", name="ytps")
                        nc.tensor.matmul(
                            yt_ps[:],
                            wo_sb[:, (2 * e2 + k) * P : (2 * e2 + k + 1) * P],
                            ot[:],
                            start=True,
                            stop=True,
                        )
                        # Pool can't read PSUM (walrus verifier), so the
                        # output copies run on DVE
                        nc.vector.tensor_copy(yt_sb[:, k, :], yt_ps[:])
                    nc.sync.dma_start(
                        yt_d[:, j * CH : (j + 1) * CH].rearrange(
                            "(e p) s -> p e s", p=P
                        )[:, 2 * e2 : 2 * e2 + 2, :],
                        yt_sb[:],
                    )

                return [(None, lambda: bc_ot(0)), (None, lambda: bc_ot(1))] + [
                    (None, lambda e2=e2: oproj(e2)) for e2 in range(ND // 2)
                ]

            # ---- startup ----
            # x chunk 0 first (its transfer overlaps the weight DMAs), then
            # warm the PE with dummy K=1 matmuls so the p-state ramp finishes
            # before the real QKV work arrives.
            xt0 = load_x(0)
            nc.sync.dma_start(wqk_sb[:], wqk_d[:])
            nc.sync.dma_start(wv_sb[:], wv_d[:])
            nc.sync.dma_start(wqkr_sb[:], wqkr_d[:])
            nc.sync.dma_start(wvr_sb[:], wvr_d[:])
            xt1 = load_x(1) if NCHUNK > 1 else None
            nc.sync.dma_start(bq_sb[:], bq_d[:])
            warm = consts.tile([1, CH], bf16)
            nc.gpsimd.memset(warm[:], 1.0)
            for w in range(18):
                wps = qyp.tile([P, CH], f32, tag="qy", name="warm")
                nc.tensor.matmul(
                    wps[:], warm[0:1, 0:P], warm[:], start=True, stop=True
                )

            qds = [None, None]
            qds[0] = qdp.tile([32 * HPC, 2, CH], f8e4, tag="qd", name="qd")
            emit_qk(0, xt0, 0, qds[0])
            emit_qk(0, xt0, 1)
            nc.sync.dma_start(tri_sb[:], tri_d[:])
            nc.sync.dma_start(onesf_sb[:], onesf_d[:])
            nc.sync.dma_start(wo_sb[:], wo_d[:])
            for st in range(4):
                filler.append((("v", 0), lambda st=st: emit_v(0, xt0, st)))
            if NCHUNK > 1:
                qds[1] = qdp.tile([32 * HPC, 2, CH], f8e4, tag="qd", name="qd")
                filler.extend(queue_qkv_fillers(1, xt1, qds[1]))

            for j in range(NCHUNK):
                drain_filler(("qk", j))  # chunk j's q/k must be in place
                qd = qds[j % 2]
                ntt = 4 * (j + 1)
                av = [
                    avp.tile([P, CH], f32, tag="av", name=f"av{h}")
                    for h in range(HPC)
                ]

                def soff(tt):
                    return (tt - 4 * j) * P if tt >= 4 * j else 0

                def issue_av(tt, pt, o):
                    for h in range(HPC):
                        nc.tensor.matmul(
                            av[h][0 : HD + 1, o:],
                            vhat[:, tt, h * (HD + 1) : (h + 1) * (HD + 1)],
                            pt[:, h, o:],
                            start=(tt == 0),
                            stop=(tt == ntt - 1),
                        )

                def flush_exp(tt, sc):
                    o = soff(tt)
                    pt = ptp.tile([P, GROUP, CH], bf16, tag="pt", name="pt")
                    sc_v = sc[:].rearrange("p (g c) -> p g c", c=CH)
                    nc.scalar.activation(
                        pt[:, :, o:], sc_v[:, :, o:], EXP, scale=0.125
                    )
                    if tt >= 4 * j:  # diagonal: mask the 128-wide triangle
                        nc.vector.tensor_mul(
                            pt[:, :, o : o + P],
                            pt[:, :, o : o + P],
                            tri_sb[:].rearrange("p (g c) -> p g c", g=1).broadcast_to(
                                [P, GROUP, P]
                            ),
                        )
                    return tt, pt, o

                # first 3 AVs of a chunk are deferred (j>0) so the PE queue
                # isn't blocked while the previous divide frees the av banks
                defer = 3 if j > 0 else 0
                av_backlog = []
                pending = None
                for g, tt in enumerate(range(ntt)):
                    if tt == 4 * j:  # diagonal flushes need chunk j's V
                        drain_filler(("v", j))
                    o = soff(tt)
                    sc = scp.tile([P, GROUP * CH], f32, tag="sc", name="sc")
                    for h in range(HPC):
                        nc.tensor.matmul(
                            sc[:, h * CH + o : (h + 1) * CH],
                            kd_sb[h * 32 : h * 32 + 32, :, tt * P : (tt + 1) * P],
                            qd[h * 32 : h * 32 + 32, :, o:CH],
                            start=True,
                            stop=True,
                            perf_mode=DR,
                        )
                    if pending is not None:
                        fl = flush_exp(*pending)
                        if len(av_backlog) < defer and g <= defer:
                            av_backlog.append(fl)
                        else:
                            if av_backlog:
                                for b in av_backlog:
                                    issue_av(*b)
                                av_backlog = []
                            issue_av(*fl)
                    pending = (tt, sc)
                    pop_filler(1 + (len(filler) > 10))
                if pending is not None:
                    fl = flush_exp(*pending)
                    for b in av_backlog:
                        issue_av(*b)
                    issue_av(*fl)

                # ---- boundary work: interleave tail of j with qkv of j+2 ----
                tail = queue_tail(j, av)
                if j + 2 < NCHUNK:
                    jn = j + 2
                    xt_t = load_x(jn)
                    dst = qdp.tile([32 * HPC, 2, CH], f8e4, tag="qd", name="qd")
                    qkv = queue_qkv_fillers(jn, xt_t, dst)
                    # order: a qk piece first (covers the recip latency), the
                    # divide, the rest of qkv, then the output projections
                    filler.append(qkv[0])
                    filler.append(tail[0])  # bc_ot(0)
                    filler.append(tail[1])  # bc_ot(1)
                    filler.extend(qkv[1:])
                    filler.extend(tail[2:])  # oproj pairs
                    qds[j % 2] = dst
                else:
                    filler.extend(tail)

            pop_filler(len(filler))

    return nc


@functools.lru_cache(maxsize=2)
def _get_nc(S):
    nc = build_nc(S)
    nc.compile()
    return nc


F8 = ml_dtypes.float8_e4m3


def _fp8_pair(a):
    """fp8 main + fp8 residual of `a` (last-axis layout unchanged)."""
    a8 = a.astype(F8)
    r8 = (a - a8.astype(np.float32)).astype(F8)
    return a8, r8


def _dr_layout(w):
    """[D, C] -> [128, NDP, 2, C]: d = (dp*2 + i)*128 + p."""
    ND = w.shape[0] // P
    c = w.shape[1]
    return np.ascontiguousarray(
        w.reshape(ND // 2, 2, P, c).transpose(2, 0, 1, 3)
    )


def make_in_maps(input, Wqkv, bqkv, Wo, S):
    """Host-side shard prep. Returns per-core input dicts."""
    x = np.asarray(input, dtype=np.float32).reshape(S, D)
    xt = np.ascontiguousarray(x.T)  # [D, S]
    Wqkv = np.asarray(Wqkv, dtype=np.float32)
    bqkv = np.asarray(bqkv, dtype=np.float32)
    Wo = np.asarray(Wo, dtype=np.float32)

    NT = S // P
    ND = D // P
    Wq, Wk, Wv = Wqkv[:, 0:D], Wqkv[:, D : 2 * D], Wqkv[:, 2 * D : 3 * D]
    bq = bqkv[0:D]

    tri = np.triu(np.ones((P, P), dtype=np.float32)).astype(BF16)  # s >= t

    # X^T in DR layout, fp8 + residual (shared by all cores)
    x8, xr8 = _fp8_pair(_dr_layout(xt))

    in_maps = []
    for c in range(NCORES):
        hs = [c * HPC + i for i in range(HPC)]
        cols = lambda W: np.concatenate(
            [W[:, h * HD : (h + 1) * HD] for h in hs], axis=1
        )  # [D, 128] in (h, d) order
        wq_l, wk_l, wv_l = cols(Wq), cols(Wk), cols(Wv)
        wqk_l = _dr_layout(np.concatenate([wq_l, wk_l], axis=1) * 32.0)
        wv_l3 = _dr_layout(wv_l * 32.0)
        wqk8, wqkr8 = _fp8_pair(wqk_l)
        wv8, wvr8 = _fp8_pair(wv_l3)
        wo_l = np.ascontiguousarray(Wo[hs[0] * HD : hs[0] * HD + HPC * HD, :]).astype(
            BF16
        )
        bq_l = np.concatenate([bq[h * HD : (h + 1) * HD] for h in hs])[:, None]
        in_maps.append(
            {
                "x8": x8,
                "xr8": xr8,
                "wqk": wqk8,
                "wqkr": wqkr8,
                "wv": wv8,
                "wvr": wvr8,
                "wo": wo_l,
                "bq": np.ascontiguousarray(bq_l, dtype=np.float32),
                "tri": tri,
                "onesb": np.ones((1, NT), dtype=BF16),
                "onesf": np.ones((1, HD), dtype=np.float32),
            }
        )
    return in_maps


def kernel(input, Wqkv, bqkv, Wo, bo):
    from concourse.bass_utils import run_bass_kernel_spmd

    S = np.asarray(input).reshape(-1, D).shape[0]
    nc = _get_nc(S)
    in_maps = make_in_maps(input, Wqkv, bqkv, Wo, S)
    res = None
    last_exc = None
    for _attempt in range(3):  # transient NRT/device errors: retry
        try:
            res = run_bass_kernel_spmd(nc, in_maps, core_ids=list(range(NCORES)))
            break
        except Exception as e:  # noqa: BLE001
            last_exc = e
    if res is None:
        raise last_exc
    yt = res.results[0]["yt"].copy()
    for r in res.results[1:]:
        yt += r["yt"]
    bqkv = np.asarray(bqkv, dtype=np.float32)
    bv = bqkv[2 * D : 3 * D]
    Wo_f = np.asarray(Wo, dtype=np.float32)
    y = yt.T + (np.asarray(bo, dtype=np.float32) + bv @ Wo_f)[None, :]
    return np.ascontiguousarray(y, dtype=np.float32).reshape(1, S, D)


# revision 55
# speedup vs baseline: 1.1205x; 1.0247x over previous
"""Multi-head causal self-attention (B=1, S=4096, D=1024, H=16) on 8 TRN2
NeuronCores, tensor-parallel over heads (2 heads per core).

v3 design (vs the fp32r baseline):
  - bf16 operands on the PE except the score matmuls, which run fp8e4m3 +
    DoubleRow (0.5 cyc/col, contraction 64 packed as [32 partitions x 2]).
  - V is produced directly in [s-part, d] orientation by swapping matmul
    operands (lhsT = X^T tile), eliminating all DVE stream-transposes.
  - k-bias dropped (softmax-invariant); v-bias folded into a host constant
    (bv @ Wo added with bo); only the q-bias is applied on device.
  - Causal masking multiplies only the 128x128 diagonal triangle per tile.
  - Softmax divide reads AV PSUM directly; denominator reciprocal feeds a
    K=1 broadcast matmul.
  - Output-projection PSUM pairs are copied to SBUF on the idle Pool engine
    and DMA'd out as fp32 partials summed on host.
  - Scheduling: per-chunk tail work (divide, oproj, next chunk's QKV) is
    split into small fillers interleaved between score/exp groups of the
    following chunk so the ACT engine (exp) never starves; the first three
    AV matmuls of each chunk are deferred so they don't head-of-line block
    the PE queue while the previous chunk's divide chain frees the AV banks.
"""

import sys

sys.path.insert(0, "/opt/trn_rl_repo")

import functools
import numpy as np
import ml_dtypes

D = 1024
H = 16
HD = 64
NCORES = 8
HPC = H // NCORES  # heads per core = 2
P = 128
CH = 512  # s-chunk width
GROUP = 2  # (tt, h0), (tt, h1) share an exp group

BF16 = ml_dtypes.bfloat16


def build_nc(S):
    import concourse.bacc as bacc
    import concourse.mybir as mybir
    from concourse import tile
    from collections import deque

    f32 = mybir.dt.float32
    f32r = mybir.dt.float32r
    bf16 = mybir.dt.bfloat16
    f8e4 = mybir.dt.float8e4
    DR = mybir.MatmulPerfMode.DoubleRow
    ADD = mybir.AluOpType.add
    MUL = mybir.AluOpType.mult
    EXP = mybir.ActivationFunctionType.Exp
    COPY = mybir.ActivationFunctionType.Copy

    NCHUNK = S // CH
    NT = S // P  # number of 128-row t-tiles
    ND = D // P  # 8 d-tiles

    nc = bacc.Bacc("TRN2", target_bir_lowering=False, debug=False)

    NDP = ND // 2  # d-tile pairs (DoubleRow contraction 256)
    # X^T and W as fp8 + fp8 residual; W pre-scaled by 32 on host, the 1/32
    # is applied when PSUM is read back
    x8_d = nc.dram_tensor("x8", [P, NDP, 2, S], f8e4, kind="ExternalInput")
    xr8_d = nc.dram_tensor("xr8", [P, NDP, 2, S], f8e4, kind="ExternalInput")
    wqk_d = nc.dram_tensor("wqk", [P, NDP, 2, 2 * P], f8e4, kind="ExternalInput")
    wqkr_d = nc.dram_tensor("wqkr", [P, NDP, 2, 2 * P], f8e4, kind="ExternalInput")
    wv_d = nc.dram_tensor("wv", [P, NDP, 2, P], f8e4, kind="ExternalInput")
    wvr_d = nc.dram_tensor("wvr", [P, NDP, 2, P], f8e4, kind="ExternalInput")
    wo_d = nc.dram_tensor("wo", [P, D], bf16, kind="ExternalInput")
    bq_d = nc.dram_tensor("bq", [P, 1], f32, kind="ExternalInput")
    tri_d = nc.dram_tensor("tri", [P, P], bf16, kind="ExternalInput")
    onesb_d = nc.dram_tensor("onesb", [1, NT], bf16, kind="ExternalInput")
    onesf_d = nc.dram_tensor("onesf", [1, HD], f32r, kind="ExternalInput")
    yt_d = nc.dram_tensor("yt", [D, S], f32, kind="ExternalOutput")

    with tile.TileContext(nc) as tc:
        with (
            tc.tile_pool(name="consts", bufs=1) as consts,
            tc.tile_pool(name="xtp", bufs=3) as xtp,
            tc.tile_pool(name="qk8p", bufs=4) as qk8p,
            tc.tile_pool(name="qdp", bufs=2) as qdp,
            tc.tile_pool(name="ptp", bufs=7) as ptp,
            tc.tile_pool(name="otp", bufs=2) as otp,
            tc.tile_pool(name="rcp", bufs=4) as rcp,
            tc.tile_pool(name="bcp", bufs=4) as bcp,
            tc.tile_pool(name="ytp", bufs=3) as ytp,
            tc.tile_pool(name="scp", bufs=2, space="PSUM") as scp,
            tc.tile_pool(name="avp", bufs=2, space="PSUM") as avp,
            tc.tile_pool(name="qyp", bufs=2, space="PSUM") as qyp,
        ):
            # ---- persistent SBUF ----
            wqk_sb = consts.tile([P, NDP, 2, 2 * P], f8e4)
            wqkr_sb = consts.tile([P, NDP, 2, 2 * P], f8e4)
            wv_sb = consts.tile([P, NDP, 2, P], f8e4)
            wvr_sb = consts.tile([P, NDP, 2, P], f8e4)
            wo_sb = consts.tile([P, D], bf16)
            bq_sb = consts.tile([P, 1], f32)
            tri_sb = consts.tile([P, P], bf16)
            onesf_sb = consts.tile([1, HD], f32r)

            # K in DoubleRow layout [32*h + d%32, d//32, t], fp8
            kd_sb = consts.tile([32 * HPC, 2, S], f8e4, name="kd")
            # bf16 q/k for chunks 0-1: their scores skip the fp8 remap wait
            kbf_sb = consts.tile([P, 2, CH], bf16, name="kbf")
            qbf_sb = [consts.tile([P, CH], bf16, name=f"qbf{t}") for t in range(2)]
            # V-hat: [t-part, NT, 2*(64+1)] cols: h0 v | ones | h1 v | ones
            vhat = consts.tile([P, NT, 2 * (HD + 1)], bf16, name="vhat")
            nc.gpsimd.memset(vhat[:, :, HD : HD + 1], 1.0)
            nc.gpsimd.memset(vhat[:, :, 2 * HD + 1 : 2 * HD + 2], 1.0)

            filler = deque()  # (key, fn) PE work bundles between groups

            def pop_filler(n=1):
                for _ in range(n):
                    if filler:
                        filler.popleft()[1]()

            def drain_filler(key):
                """Run all queued fillers up to and including the last one
                tagged `key` (chunk-j qkv work chunk j depends on)."""
                while any(k == key for k, _ in filler):
                    pop_filler()

            def load_x(j):
                """Two DMAs for the chunk's fp8 X^T + residual."""
                x8_t = xtp.tile([P, NDP, 2, CH], f8e4, tag="xt", name="x8_t")
                xr8_t = xtp.tile([P, NDP, 2, CH], f8e4, tag="xt", name="xr8_t")
                nc.sync.dma_start(x8_t[:], x8_d[:, :, :, j * CH : (j + 1) * CH])
                nc.sync.dma_start(xr8_t[:], xr8_d[:, :, :, j * CH : (j + 1) * CH])
                return (x8_t, xr8_t)

            def qk_mm(xt_t, c, ps, dlo, dhi):
                """3-chain compensated fp8: x8@w8 + xr8@w8 + x8@wr8."""
                x8_t, xr8_t = xt_t
                chains = [(wqk_sb, x8_t), (wqk_sb, xr8_t), (wqkr_sb, x8_t)]
                for d in range(dlo, dhi):
                    for ci, (w, x) in enumerate(chains):
                        nc.tensor.matmul(
                            ps[:],
                            w[:, d, :, c * P : (c + 1) * P],
                            x[:, d, :, :],
                            start=(d == 0 and ci == 0),
                            stop=(d == NDP - 1 and ci == len(chains) - 1),
                            perf_mode=DR,
                        )

            def qk_fin(j, c, ps, qd_dst=None):
                """fp8 conversion (1/32 weight scale, + q bias) and remap."""
                def scale_to(dst):
                    if c == 0:
                        nc.vector.tensor_scalar(
                            out=dst, in0=ps[:], scalar1=1.0 / 32,
                            scalar2=bq_sb[:, 0:1], op0=MUL, op1=ADD,
                        )
                    else:
                        nc.vector.tensor_scalar(
                            out=dst, in0=ps[:], scalar1=1.0 / 32,
                            scalar2=None, op0=MUL,
                        )

                if j < 2:  # bf16 fast path for the startup chunks' scores
                    scale_to(qbf_sb[j][:] if c == 0 else kbf_sb[:, j, :])
                    if c == 0:
                        return  # fp8 q of chunks 0-1 is never read
                q8 = qk8p.tile([P, CH], f8e4, tag="qk8", name=f"qk8{c}")
                scale_to(q8[:])
                if c == 0:
                    dcol = qd_dst[:, :, 0:CH]
                else:
                    dcol = kd_sb[:, :, j * CH : (j + 1) * CH]
                # 4 plain rectangles: a multi-partition-dim source AP
                # lowers to garbage on HW
                for h in range(HPC):
                    for i in range(2):
                        nc.sync.dma_start(
                            dcol[h * 32 : h * 32 + 32, i, :],
                            q8[h * HD + i * 32 : h * HD + i * 32 + 32, :],
                        )

            def emit_qk(j, xt_t, c, qd_dst=None):
                ps = qyp.tile([P, CH], f32, tag="qy", name=f"qkps{c}")
                qk_mm(xt_t, c, ps, 0, NDP)
                qk_fin(j, c, ps, qd_dst)

            def emit_v(j, xt_t, st):
                """V in direct [s, d] orientation for one 128-row s-tile."""
                x8_t, xr8_t = xt_t
                vps = qyp.tile([P, P], f32, tag="qy", name="vps")
                chains = [(x8_t, wv_sb), (xr8_t, wv_sb), (x8_t, wvr_sb)]
                for d in range(NDP):
                    for ci, (x, w) in enumerate(chains):
                        nc.tensor.matmul(
                            vps[:],
                            x[:, d, :, st * P : (st + 1) * P],
                            w[:, d, :, :],
                            start=(d == 0 and ci == 0),
                            stop=(d == NDP - 1 and ci == len(chains) - 1),
                            perf_mode=DR,
                        )
                vh = vhat[:].rearrange("p t (h c) -> p t h c", h=2)
                nc.vector.tensor_scalar(
                    out=vh[:, 4 * j + st, :, 0:HD],
                    in0=vps[:].rearrange("p (h c) -> p h c", h=2),
                    scalar1=1.0 / 32,
                    scalar2=None,
                    op0=MUL,
                )

            def queue_qkv_fillers(j, xt_t, dst):
                """Fine-grained (key, fn) fillers (~0.5-0.9us PE work each)."""
                out = []
                for c in range(2):
                    ps = qyp.tile([P, CH], f32, tag="qy", name=f"qkps{c}")
                    out.append(
                        (("qk", j), lambda c=c, ps=ps: qk_mm(xt_t, c, ps, 0, 2))
                    )
                    out.append(
                        (
                            ("qk", j),
                            lambda c=c, ps=ps: (
                                qk_mm(xt_t, c, ps, 2, NDP),
                                qk_fin(j, c, ps, dst),
                            ),
                        )
                    )
                for st in range(4):
                    out.append((("v", j), lambda st=st: emit_v(j, xt_t, st)))
                return out

            def queue_tail(j, av):
                """divide + output projection of chunk j as fillers.

                Caller interleaves these with the next chunk's qkv fillers.
                """
                rcs, nms = [], []
                for h in range(HPC):
                    rc = rcp.tile([1, CH], f32r, tag="rc", name="rc")
                    with nc.allow_low_precision("recip feeds fp22 matmul"):
                        nc.vector.reciprocal(rc[:], av[h][HD : HD + 1, :])
                    rcs.append(rc)
                    # numerator copy also frees the av bank for the next chunk
                    nm = bcp.tile([HD, CH], bf16, tag="nm", name="nm")
                    nc.vector.tensor_copy(nm[:], av[h][0:HD, :])
                    nms.append(nm)
                ot = otp.tile([P, CH], bf16, tag="ot", name="ot")

                def bc_ot(h):
                    bc = qyp.tile([P, CH], f32, tag="qy", name="bc")
                    nc.tensor.matmul(
                        bc[0:HD, :], onesf_sb[0:1, :], rcs[h][:],
                        start=True, stop=True,
                    )
                    nc.vector.tensor_mul(
                        ot[h * HD : (h + 1) * HD, :],
                        nms[h][:],
                        bc[0:HD, :],
                    )

                last = j == NCHUNK - 1

                def oproj(e2):
                    yt_sb = ytp.tile([P, 2, CH], f32, tag="yt", name="ytsb")
                    for k in range(2):
                        yt_ps = qyp.tile([P, CH], f32, tag="qy", name="ytps")
                        nc.tensor.matmul(
                            yt_ps[:],
                            wo_sb[:, (2 * e2 + k) * P : (2 * e2 + k + 1) * P],
                            ot[:],
                            start=True,
                            stop=True,
                        )
                        # Pool can't read PSUM (walrus verifier): copies run
                        # on DVE; in the final drain ACT (idle) takes half
                        if last and (e2 + k) % 2:
                            nc.scalar.activation(
                                yt_sb[:, k, :], yt_ps[:], COPY, scale=1.0
                            )
                        else:
                            nc.vector.tensor_copy(yt_sb[:, k, :], yt_ps[:])
                    nc.sync.dma_start(
                        yt_d[:, j * CH : (j + 1) * CH].rearrange(
                            "(e p) s -> p e s", p=P
                        )[:, 2 * e2 : 2 * e2 + 2, :],
                        yt_sb[:],
                    )

                return [(None, lambda: bc_ot(0)), (None, lambda: bc_ot(1))] + [
                    (None, lambda e2=e2: oproj(e2)) for e2 in range(ND // 2)
                ]

            # ---- startup ----
            # x chunk 0 first (its transfer overlaps the weight DMAs), then
            # warm the PE with dummy K=1 matmuls so the p-state ramp finishes
            # before the real QKV work arrives.
            xt0 = load_x(0)
            nc.sync.dma_start(wqk_sb[:], wqk_d[:])
            nc.sync.dma_start(wv_sb[:], wv_d[:])
            nc.sync.dma_start(wqkr_sb[:], wqkr_d[:])
            nc.sync.dma_start(wvr_sb[:], wvr_d[:])
            xt1 = load_x(1) if NCHUNK > 1 else None
            nc.sync.dma_start(bq_sb[:], bq_d[:])
            warm = consts.tile([1, CH], bf16)
            nc.gpsimd.memset(warm[:], 1.0)
            for w in range(8):
                wps = qyp.tile([P, CH], f32, tag="qy", name="warm")
                nc.tensor.matmul(
                    wps[:], warm[0:1, 0:P], warm[:], start=True, stop=True
                )

            qds = [None, None]
            emit_qk(0, xt0, 0, None)  # chunks 0-1 score via the bf16 path
            emit_qk(0, xt0, 1)
            nc.sync.dma_start(tri_sb[:], tri_d[:])
            nc.sync.dma_start(onesf_sb[:], onesf_d[:])
            nc.sync.dma_start(wo_sb[:], wo_d[:])
            for st in range(4):
                filler.append((("v", 0), lambda st=st: emit_v(0, xt0, st)))
            if NCHUNK > 1:
                filler.extend(queue_qkv_fillers(1, xt1, None))

            for j in range(NCHUNK):
                drain_filler(("qk", j))  # chunk j's q/k must be in place
                qd = qds[j % 2]
                ntt = 4 * (j + 1)
                av = [
                    avp.tile([P, CH], f32, tag="av", name=f"av{h}")
                    for h in range(HPC)
                ]

                def soff(tt):
                    return (tt - 4 * j) * P if tt >= 4 * j else 0

                def issue_av(tt, pt, o):
                    if tt == 4 * j:  # diagonal AVs need chunk j's V
                        drain_filler(("v", j))
                    for h in range(HPC):
                        nc.tensor.matmul(
                            av[h][0 : HD + 1, o:],
                            vhat[:, tt, h * (HD + 1) : (h + 1) * (HD + 1)],
                            pt[:, h, o:],
                            start=(tt == 0),
                            stop=(tt == ntt - 1),
                        )

                def flush_exp(tt, sc):
                    o = soff(tt)
                    pt = ptp.tile([P, GROUP, CH], bf16, tag="pt", name="pt")
                    sc_v = sc[:].rearrange("p (g c) -> p g c", c=CH)
                    nc.scalar.activation(
                        pt[:, :, o:], sc_v[:, :, o:], EXP, scale=0.125
                    )
                    if tt >= 4 * j:  # diagonal: mask the 128-wide triangle
                        nc.vector.tensor_mul(
                            pt[:, :, o : o + P],
                            pt[:, :, o : o + P],
                            tri_sb[:].rearrange("p (g c) -> p g c", g=1).broadcast_to(
                                [P, GROUP, P]
                            ),
                        )
                    return tt, pt, o

                # first 3 AVs of a chunk are deferred (j>0) so the PE queue
                # isn't blocked while the previous divide frees the av banks
                defer = 3 if j > 0 else 0
                av_backlog = []
                pending = None
                for g, tt in enumerate(range(ntt)):
                    o = soff(tt)
                    sc = scp.tile([P, GROUP * CH], f32, tag="sc", name="sc")
                    for h in range(HPC):
                        if j < 2:  # bf16 startup path (no fp8 remap wait)
                            jt, tl = divmod(tt, 4)
                            nc.tensor.matmul(
                                sc[:, h * CH + o : (h + 1) * CH],
                                kbf_sb[
                                    h * HD : (h + 1) * HD, jt,
                                    tl * P : (tl + 1) * P,
                                ],
                                qbf_sb[j][h * HD : (h + 1) * HD, o:CH],
                                start=True,
                                stop=True,
                            )
                        else:
                            nc.tensor.matmul(
                                sc[:, h * CH + o : (h + 1) * CH],
                                kd_sb[h * 32 : h * 32 + 32, :, tt * P : (tt + 1) * P],
                                qd[h * 32 : h * 32 + 32, :, o:CH],
                                start=True,
                                stop=True,
                                perf_mode=DR,
                            )
                    if pending is not None:
                        fl = flush_exp(*pending)
                        if len(av_backlog) < defer and g <= defer:
                            av_backlog.append(fl)
                        else:
                            if av_backlog:
                                for b in av_backlog:
                                    issue_av(*b)
                                av_backlog = []
                            issue_av(*fl)
                    pending = (tt, sc)
                    pop_filler(1 + (len(filler) > 10))
                if pending is not None:
                    fl = flush_exp(*pending)
                    for b in av_backlog:
                        issue_av(*b)
                    issue_av(*fl)

                # ---- boundary work: interleave tail of j with qkv of j+2 ----
                tail = queue_tail(j, av)
                if j + 2 < NCHUNK:
                    jn = j + 2
                    xt_t = load_x(jn)
                    dst = qdp.tile([32 * HPC, 2, CH], f8e4, tag="qd", name="qd")
                    qkv = queue_qkv_fillers(jn, xt_t, dst)
                    # order: a qk piece first (covers the recip latency), the
                    # divide, the rest of qkv, then the output projections
                    filler.append(qkv[0])
                    filler.append(tail[0])  # bc_ot(0)
                    filler.append(tail[1])  # bc_ot(1)
                    filler.extend(qkv[1:])
                    filler.extend(tail[2:])  # oproj pairs
                    qds[j % 2] = dst
                else:
                    filler.extend(tail)

            pop_filler(len(filler))

    return nc


@functools.lru_cache(maxsize=2)
def _get_nc(S):
    nc = build_nc(S)
    nc.compile()
    return nc


F8 = ml_dtypes.float8_e4m3


def _fp8_pair(a):
    """fp8 main + fp8 residual of `a` (last-axis layout unchanged)."""
    a8 = a.astype(F8)
    r8 = (a - a8.astype(np.float32)).astype(F8)
    return a8, r8


def _dr_layout(w):
    """[D, C] -> [128, NDP, 2, C]: d = (dp*2 + i)*128 + p."""
    ND = w.shape[0] // P
    c = w.shape[1]
    return np.ascontiguousarray(
        w.reshape(ND // 2, 2, P, c).transpose(2, 0, 1, 3)
    )


def make_in_maps(input, Wqkv, bqkv, Wo, S):
    """Host-side shard prep. Returns per-core input dicts."""
    x = np.asarray(input, dtype=np.float32).reshape(S, D)
    xt = np.ascontiguousarray(x.T)  # [D, S]
    Wqkv = np.asarray(Wqkv, dtype=np.float32)
    bqkv = np.asarray(bqkv, dtype=np.float32)
    Wo = np.asarray(Wo, dtype=np.float32)

    NT = S // P
    ND = D // P
    Wq, Wk, Wv = Wqkv[:, 0:D], Wqkv[:, D : 2 * D], Wqkv[:, 2 * D : 3 * D]
    bq = bqkv[0:D]

    tri = np.triu(np.ones((P, P), dtype=np.float32)).astype(BF16)  # s >= t

    # X^T in DR layout, fp8 + residual (shared by all cores)
    x8, xr8 = _fp8_pair(_dr_layout(xt))

    in_maps = []
    for c in range(NCORES):
        hs = [c * HPC + i for i in range(HPC)]
        cols = lambda W: np.concatenate(
            [W[:, h * HD : (h + 1) * HD] for h in hs], axis=1
        )  # [D, 128] in (h, d) order
        wq_l, wk_l, wv_l = cols(Wq), cols(Wk), cols(Wv)
        wqk_l = _dr_layout(np.concatenate([wq_l, wk_l], axis=1) * 32.0)
        wv_l3 = _dr_layout(wv_l * 32.0)
        wqk8, wqkr8 = _fp8_pair(wqk_l)
        wv8, wvr8 = _fp8_pair(wv_l3)
        wo_l = np.ascontiguousarray(Wo[hs[0] * HD : hs[0] * HD + HPC * HD, :]).astype(
            BF16
        )
        bq_l = np.concatenate([bq[h * HD : (h + 1) * HD] for h in hs])[:, None]
        in_maps.append(
            {
                "x8": x8,
                "xr8": xr8,
                "wqk": wqk8,
                "wqkr": wqkr8,
                "wv": wv8,
                "wvr": wvr8,
                "wo": wo_l,
                "bq": np.ascontiguousarray(bq_l, dtype=np.float32),
                "tri": tri,
                "onesb": np.ones((1, NT), dtype=BF16),
                "onesf": np.ones((1, HD), dtype=np.float32),
            }
        )
    return in_maps


def kernel(input, Wqkv, bqkv, Wo, bo):
    from concourse.bass_utils import run_bass_kernel_spmd

    S = np.asarray(input).reshape(-1, D).shape[0]
    nc = _get_nc(S)
    in_maps = make_in_maps(input, Wqkv, bqkv, Wo, S)
    res = None
    last_exc = None
    for _attempt in range(3):  # transient NRT/device errors: retry
        try:
            res = run_bass_kernel_spmd(nc, in_maps, core_ids=list(range(NCORES)))
            break
        except Exception as e:  # noqa: BLE001
            last_exc = e
    if res is None:
        raise last_exc
    yt = res.results[0]["yt"].copy()
    for r in res.results[1:]:
        yt += r["yt"]
    bqkv = np.asarray(bqkv, dtype=np.float32)
    bv = bqkv[2 * D : 3 * D]
    Wo_f = np.asarray(Wo, dtype=np.float32)
    y = yt.T + (np.asarray(bo, dtype=np.float32) + bv @ Wo_f)[None, :]
    return np.ascontiguousarray(y, dtype=np.float32).reshape(1, S, D)
